# revision 36
# baseline (speedup 1.0000x reference)
"""MoE all-to-all token dispatch kernel for 8 Trainium2 NeuronCores.

Problem: out[d, t*K+k, :] = x[t, :] if expert_mapping[expert_indices[t, k]] == d
else 0, with B=4, S=4096, H=512, K=2, 64 experts, 8 devices.

Strategy: the output's leading device axis is sharded across the 8 cores —
core d produces out[d] = [T*K, H].  Only ~1/8 of each core's output rows are
nonzero, so each core gathers just the needed token rows from HBM into SBUF
(extended dma_gather ucode, 512-row groups) and scatter-adds them into the
owned slots of its runtime pre-zeroed output (dma_scatter_add; pad slots add
zero rows to distinct unowned output rows, so the static instruction stream
is identical on every core).

The token payload travels as fp16 END TO END: the device output buffer is
fp16 and the host upcasts to fp32 during final assembly (MoE dispatch in
16-bit is standard practice; the harness gate is rel_err < 2e-2 and the
fp16 round-trip is ~4e-4).  This halves both the gather reads (1KB packets,
~55ns) and the scatter read-modify-writes, dropping total DMA engine time
well under the GPSIMD descriptor-generation time (~8ns/row for gather ucode
+ ~1.5ns/row for scatter_add), which becomes the pipeline's critical path.

Load balancing is 128-row granular: all cores run an identical instruction
stream of nch chunk-units targeting their own `out` tensor.  Slabs larger
than nch*128 export 128-row chunks into other cores' spare chunk slots;
because output-row ownership is a partition, exported rows never collide
with the hosting core's own rows, and the host stitches them back
(re-zeroing them on the hosting core's slab) during final assembly.

Index tensors load via the Sync engine's HWDGE, overlapping the ~11us
GPSIMD ucode library load + first-use IRAM fetch.
"""

import numpy as np

B, S, H, K = 4, 4096, 512, 2
T = B * S          # 16384 tokens
TK = T * K         # 32768 output rows per device
D = 8              # devices / NeuronCores
E = 64             # experts

ZPAD = 128         # appended all-zero rows in xin (pad-slot gather targets)
ZROW = T           # index of the first zero row
CH = 128           # slots per chunk-unit (balancing granularity)
GRP = 4            # chunks per full gather/scatter group (512 rows)
LEAD = 2           # leading groups halved (256 rows) to start scatters early
IND_CH = 0         # trailing chunks scattered via indirect pure writes

TRACE = False
LAST_EXEC_NS = None
LAST_RESULTS = None

_CACHE = {}


def _wrap_idxs16(vals: np.ndarray) -> np.ndarray:
    """Extended-instruction SWDGE wrapped int16 layout: element i at
    [i % 16, i // 16], replicated across the 8 partition groups."""
    n = len(vals)
    assert n % 16 == 0
    w = vals.astype(np.int16).reshape(n // 16, 16).T      # [16, n/16]
    return np.ascontiguousarray(np.tile(w, (8, 1)))       # [128, n/16]


def _group_plan(nch: int):
    """Group chunk-units for the pipeline: LEAD leading half-groups (so the
    scatter stream starts early), full GRP-chunk groups after, and the last
    IND_CH chunks scattered via indirect pure writes instead of
    dma_scatter_add.  Returns (groups, n_add_ch) where each group is
    (c0, gsz, mech, a_of/i_of offset within its mechanism class)."""
    n_ind = min(IND_CH, nch)
    n_add = nch - n_ind
    sizes = []
    c = 0
    while c < n_add:
        gsz = min(GRP // 2 if len(sizes) < LEAD else GRP, n_add - c)
        sizes.append(("a", gsz))
        c += gsz
    while c < nch:
        gsz = min(GRP, nch - c)
        sizes.append(("i", gsz))
        c += gsz
    groups = []
    c = a_cum = i_cum = 0
    for mech, gsz in sizes:
        groups.append((c, gsz, mech, a_cum if mech == "a" else i_cum))
        if mech == "a":
            a_cum += gsz
        else:
            i_cum += gsz
        c += gsz
    return groups, n_add


def _build_module(nch: int):
    from contextlib import ExitStack

    import concourse.bacc as bacc
    import concourse.bass as bass
    import concourse.mybir as mybir
    from concourse.library_config import mlp

    maxn = nch * CH
    groups, n_add_ch = _group_plan(nch)
    ng = len(groups)
    n_ind_ch = nch - n_add_ch

    nc = bacc.Bacc("TRN2", debug=False, num_swdge_queues=4,
                   dynamic_dma_scratch_size=65536)
    xin = nc.dram_tensor("xin", [T + ZPAD, H], mybir.dt.float16,
                         kind="ExternalInput")
    sidx = nc.dram_tensor("sidx", [128, maxn // 16], mybir.dt.int16,
                          kind="ExternalInput")
    didx = nc.dram_tensor("didx", [128, max(n_add_ch * 8, 16)],
                          mybir.dt.int16, kind="ExternalInput")
    didx_i = nc.dram_tensor("didx_i", [128, max(n_ind_ch, 1)],
                            mybir.dt.int32, kind="ExternalInput")
    out = nc.dram_tensor("out", [TK, H], mybir.dt.float16,
                         kind="ExternalOutput")

    with (
        nc.Block() as block,
        nc.sbuf_tensor("data16", [128, nch, H], mybir.dt.float16) as data16,
        nc.sbuf_tensor("sidx_sb", [128, maxn // 16], mybir.dt.int16)
        as sidx_sb,
        nc.sbuf_tensor("didx_sb", [128, max(n_add_ch * 8, 16)],
                       mybir.dt.int16) as didx_sb,
        nc.sbuf_tensor("didx_i_sb", [128, max(n_ind_ch, 1)],
                       mybir.dt.int32) as didx_i_sb,
        nc.semaphore("io0") as io0,
        nc.semaphore("ssem") as ssem,
        ExitStack() as stack,
    ):
        gsems = [stack.enter_context(nc.semaphore(f"g{g}"))  # noqa: ANT232
                 for g in range(ng)]
        LOOK = 4

        @block.sync
        def _(sync):
            # HWDGE loads overlap GPSIMD's ucode library load
            sync.dma_start(sidx_sb[:], sidx[:]).then_inc(io0, 16)
            sync.dma_start(didx_sb[:], didx[:]).then_inc(io0, 16)
            sync.dma_start(didx_i_sb[:], didx_i[:]).then_inc(io0, 16)

        @block.gpsimd
        def _(gpsimd):
            gpsimd.load_library(mlp)

            def gather(g):
                c0, gsz, _, _ = groups[g]
                gpsimd.dma_gather(
                    data16[:, c0:c0 + gsz, :], xin[:],
                    sidx_sb[:, c0 * 8:(c0 + gsz) * 8], gsz * CH, gsz * CH,
                    H, single_packet=True, queue_num=g % 2,
                ).then_inc(gsems[g], 16)

            gpsimd.wait_ge(io0, 48)
            for g in range(min(LOOK, ng)):
                gather(g)
            n_sc = 0
            for g, (c0, gsz, mech, mof) in enumerate(groups):
                gpsimd.wait_ge(gsems[g], 16)
                if mech == "a":
                    gpsimd.dma_scatter_add(
                        out[:], data16[:, c0:c0 + gsz, :],
                        didx_sb[:, mof * 8:(mof + gsz) * 8],
                        gsz * CH, gsz * CH, H,
                        single_packet=False, queue_num=2 + g % 2,
                    ).then_inc(ssem, 16)
                    n_sc += 1
                else:
                    for k in range(gsz):
                        gpsimd.indirect_dma_start(
                            out=out[:],
                            out_offset=bass.IndirectOffsetOnAxis(
                                ap=didx_i_sb[:, mof + k:mof + k + 1],
                                axis=0),
                            in_=data16[:, c0 + k:c0 + k + 1, :].squeeze(1),
                            in_offset=None,
                        ).then_inc(ssem, 16)
                        n_sc += 1
                if g + LOOK < ng:
                    gather(g + LOOK)
            gpsimd.wait_ge(ssem, 16 * n_sc)

    nc.compile()
    return nc


def kernel(input_tensor, expert_indices, expert_mapping):
    global LAST_EXEC_NS, LAST_RESULTS
    from concourse.bass_utils import run_bass_kernel_spmd

    x = np.zeros((T + ZPAD, H), dtype=np.float16)
    x[:T] = np.asarray(input_tensor, dtype=np.float32).reshape(
        T, H).astype(np.float16)
    idx = np.asarray(expert_indices, dtype=np.int32).reshape(-1)
    emap = np.asarray(expert_mapping, dtype=np.int32)
    owner = emap[idx]                                  # [T*K], slot r = t*K+k

    dsts = [np.nonzero(owner == d)[0] for d in range(D)]
    sizes = [len(v) for v in dsts]

    # Smallest uniform per-core chunk count nch such that every slab's
    # overflow (in 128-row export chunks) fits into other cores' spare
    # chunk slots.
    nch = -(-max(TK // D, max(sizes)) // CH)
    for cand in range(-(-(TK // D) // CH), nch + 1):
        spare = sum(max(0, cand - (-(-min(s, cand * CH) // CH)))
                    for s in sizes)
        exp = sum(-(-max(0, s - cand * CH) // CH) for s in sizes)
        if spare >= exp:
            nch = cand
            break
    maxn = nch * CH

    kept = [dsts[d][: min(sizes[d], maxn)] for d in range(D)]
    exports = []                       # (owner, rows) in 128-row chunks
    for d in range(D):
        rest = dsts[d][maxn:]
        for lo in range(0, len(rest), CH):
            exports.append((d, rest[lo: lo + CH]))

    # Assign export chunks to cores with spare chunk slots.  Core 0 hosts
    # only if the others can't absorb everything (trailing-pad slots are
    # skipped via negative indices, so unused spare is free).
    spare_of = [nch - (-(-len(kept[d]) // CH)) for d in range(D)]
    hosted = [[] for _ in range(D)]    # per host core: list of (owner, rows)
    order = sorted(range(1, D), key=lambda d: -spare_of[d]) + [0]
    pos = 0
    for exp in exports:
        while spare_of[order[pos]] - len(hosted[order[pos]]) <= 0:
            pos += 1
        hosted[order[pos]].append(exp)

    if nch not in _CACHE:
        _CACHE[nch] = _build_module(nch)
    nc = _CACHE[nch]

    in_maps = []
    for d in range(D):
        forbid = np.zeros(TK, bool)
        forbid[kept[d]] = True
        for o, rows in hosted[d]:
            forbid[rows] = True
        free_rows = np.nonzero(~forbid)[0]

        # slot sequence: own rows (tail-padded to a chunk boundary), then
        # each hosted export chunk (padded), then all-pad chunks.
        seq_s, seq_t = [], []
        fpos = 0
        seq_s.append(kept[d] // K)
        seq_t.append(kept[d])
        total = len(kept[d])
        if total % CH:
            npad_c = CH - total % CH
            seq_s.append(ZROW + (np.arange(npad_c) % ZPAD))
            seq_t.append(free_rows[fpos:fpos + npad_c])
            fpos += npad_c
            total += npad_c
        for o, rows in hosted[d]:
            seq_s.append(rows // K)
            seq_t.append(rows)
            total += len(rows)
            if len(rows) % CH:
                npad_c = CH - len(rows) % CH
                seq_s.append(ZROW + (np.arange(npad_c) % ZPAD))
                seq_t.append(free_rows[fpos:fpos + npad_c])
                fpos += npad_c
                total += npad_c
        if total < maxn:
            nrest = maxn - total
            seq_s.append(ZROW + (np.arange(nrest) % ZPAD))
            seq_t.append(free_rows[fpos:fpos + nrest])
            fpos += nrest
        srcfull = np.concatenate(seq_s)
        dstfull = np.concatenate(seq_t)
        assert len(srcfull) == maxn

        _, n_add_ch = _group_plan(nch)
        n_ind_ch = nch - n_add_ch
        in_maps.append({
            "xin": x,
            "sidx": _wrap_idxs16(srcfull),
            "didx": _wrap_idxs16(dstfull[:n_add_ch * CH])
            if n_add_ch else np.zeros((128, 16), np.int16),
            "didx_i": np.ascontiguousarray(
                dstfull[n_add_ch * CH:].astype(np.int32).reshape(
                    n_ind_ch, CH).T)
            if n_ind_ch else np.zeros((128, 1), np.int32),
        })

    res = run_bass_kernel_spmd(nc, in_maps, list(range(D)), trace=TRACE)
    if TRACE:
        LAST_EXEC_NS = res.exec_time_ns
        LAST_RESULTS = res
    outs = [np.array(res.results[d]["out"]).astype(np.float32)
            for d in range(D)]
    for c in range(D):
        for o, rows in hosted[c]:
            outs[o][rows] = np.asarray(
                res.results[c]["out"][rows], dtype=np.float32)
            outs[c][rows] = 0.0
    return np.stack(outs, axis=0)


# revision 37
# speedup vs baseline: 1.0087x; 1.0087x over previous
"""MoE all-to-all token dispatch kernel for 8 Trainium2 NeuronCores.

Problem: out[d, t*K+k, :] = x[t, :] if expert_mapping[expert_indices[t, k]] == d
else 0, with B=4, S=4096, H=512, K=2, 64 experts, 8 devices.

Strategy: the output's leading device axis is sharded across the 8 cores —
core d produces out[d] = [T*K, H].  Only ~1/8 of each core's output rows are
nonzero, so each core gathers just the needed token rows from HBM into SBUF
(extended dma_gather ucode, 512-row groups) and scatter-adds them into the
owned slots of its runtime pre-zeroed output (dma_scatter_add; pad slots add
zero rows to distinct unowned output rows, so the static instruction stream
is identical on every core).

The token payload travels as fp16 END TO END: the device output buffer is
fp16 and the host upcasts to fp32 during final assembly (MoE dispatch in
16-bit is standard practice; the harness gate is rel_err < 2e-2 and the
fp16 round-trip is ~4e-4).  This halves both the gather reads (1KB packets,
~55ns) and the scatter read-modify-writes, dropping total DMA engine time
well under the GPSIMD descriptor-generation time (~8ns/row for gather ucode
+ ~1.5ns/row for scatter_add), which becomes the pipeline's critical path.

Load balancing is 128-row granular: all cores run an identical instruction
stream of nch chunk-units targeting their own `out` tensor.  Slabs larger
than nch*128 export 128-row chunks into other cores' spare chunk slots;
because output-row ownership is a partition, exported rows never collide
with the hosting core's own rows, and the host stitches them back
(re-zeroing them on the hosting core's slab) during final assembly.

Index tensors load via the Sync engine's HWDGE, overlapping the ~11us
GPSIMD ucode library load + first-use IRAM fetch.
"""

import numpy as np

B, S, H, K = 4, 4096, 512, 2
T = B * S          # 16384 tokens
TK = T * K         # 32768 output rows per device
D = 8              # devices / NeuronCores
E = 64             # experts

ZPAD = 128         # appended all-zero rows in xin (pad-slot gather targets)
ZROW = T           # index of the first zero row
CH = 128           # slots per chunk-unit (balancing granularity)
GRP = 4            # chunks per full gather/scatter group (512 rows)
LEAD = 2           # leading groups halved (256 rows) to start scatters early
IND_CH = 0         # trailing chunks scattered via indirect pure writes

TRACE = False
LAST_EXEC_NS = None
LAST_RESULTS = None

_CACHE = {}


def _wrap_idxs16(vals: np.ndarray) -> np.ndarray:
    """Extended-instruction SWDGE wrapped int16 layout: element i at
    [i % 16, i // 16], replicated across the 8 partition groups."""
    n = len(vals)
    assert n % 16 == 0
    w = vals.astype(np.int16).reshape(n // 16, 16).T      # [16, n/16]
    return np.ascontiguousarray(np.tile(w, (8, 1)))       # [128, n/16]


def _group_plan(nch: int):
    """Group chunk-units for the pipeline: LEAD leading half-groups (so the
    scatter stream starts early), full GRP-chunk groups after, and the last
    IND_CH chunks scattered via indirect pure writes instead of
    dma_scatter_add.  Returns (groups, n_add_ch) where each group is
    (c0, gsz, mech, a_of/i_of offset within its mechanism class)."""
    n_ind = min(IND_CH, nch)
    n_add = nch - n_ind
    sizes = []
    c = 0
    while c < n_add:
        gsz = min(GRP // 2 if len(sizes) < LEAD else GRP, n_add - c)
        sizes.append(("a", gsz))
        c += gsz
    while c < nch:
        gsz = min(GRP, nch - c)
        sizes.append(("i", gsz))
        c += gsz
    groups = []
    c = a_cum = i_cum = 0
    for mech, gsz in sizes:
        groups.append((c, gsz, mech, a_cum if mech == "a" else i_cum))
        if mech == "a":
            a_cum += gsz
        else:
            i_cum += gsz
        c += gsz
    return groups, n_add


def _build_module(nch: int):
    from contextlib import ExitStack

    import concourse.bacc as bacc
    import concourse.bass as bass
    import concourse.mybir as mybir
    from concourse.library_config import mlp

    maxn = nch * CH
    groups, n_add_ch = _group_plan(nch)
    ng = len(groups)
    n_ind_ch = nch - n_add_ch

    nc = bacc.Bacc("TRN2", debug=False, num_swdge_queues=4,
                   dynamic_dma_scratch_size=49152)
    xin = nc.dram_tensor("xin", [T + ZPAD, H], mybir.dt.float16,
                         kind="ExternalInput")
    sidx = nc.dram_tensor("sidx", [128, maxn // 16], mybir.dt.int16,
                          kind="ExternalInput")
    didx = nc.dram_tensor("didx", [128, max(n_add_ch * 8, 16)],
                          mybir.dt.int16, kind="ExternalInput")
    didx_i = nc.dram_tensor("didx_i", [128, max(n_ind_ch, 1)],
                            mybir.dt.int32, kind="ExternalInput")
    out = nc.dram_tensor("out", [TK, H], mybir.dt.float16,
                         kind="ExternalOutput")

    with (
        nc.Block() as block,
        nc.sbuf_tensor("data16", [128, nch, H], mybir.dt.float16) as data16,
        nc.sbuf_tensor("sidx_sb", [128, maxn // 16], mybir.dt.int16)
        as sidx_sb,
        nc.sbuf_tensor("didx_sb", [128, max(n_add_ch * 8, 16)],
                       mybir.dt.int16) as didx_sb,
        nc.sbuf_tensor("didx_i_sb", [128, max(n_ind_ch, 1)],
                       mybir.dt.int32) as didx_i_sb,
        nc.semaphore("io0") as io0,
        nc.semaphore("ssem") as ssem,
        ExitStack() as stack,
    ):
        gsems = [stack.enter_context(nc.semaphore(f"g{g}"))  # noqa: ANT232
                 for g in range(ng)]
        LOOK = 3

        @block.sync
        def _(sync):
            # HWDGE loads overlap GPSIMD's ucode library load
            sync.dma_start(sidx_sb[:], sidx[:]).then_inc(io0, 16)
            sync.dma_start(didx_sb[:], didx[:]).then_inc(io0, 16)
            sync.dma_start(didx_i_sb[:], didx_i[:]).then_inc(io0, 16)

        @block.gpsimd
        def _(gpsimd):
            gpsimd.load_library(mlp)

            def gather(g):
                c0, gsz, _, _ = groups[g]
                gpsimd.dma_gather(
                    data16[:, c0:c0 + gsz, :], xin[:],
                    sidx_sb[:, c0 * 8:(c0 + gsz) * 8], gsz * CH, gsz * CH,
                    H, single_packet=True, queue_num=g % 2,
                ).then_inc(gsems[g], 16)

            gpsimd.wait_ge(io0, 48)
            for g in range(min(LOOK, ng)):
                gather(g)
            n_sc = 0
            for g, (c0, gsz, mech, mof) in enumerate(groups):
                gpsimd.wait_ge(gsems[g], 16)
                if mech == "a":
                    gpsimd.dma_scatter_add(
                        out[:], data16[:, c0:c0 + gsz, :],
                        didx_sb[:, mof * 8:(mof + gsz) * 8],
                        gsz * CH, gsz * CH, H,
                        single_packet=False, queue_num=2 + g % 2,
                    ).then_inc(ssem, 16)
                    n_sc += 1
                else:
                    for k in range(gsz):
                        gpsimd.indirect_dma_start(
                            out=out[:],
                            out_offset=bass.IndirectOffsetOnAxis(
                                ap=didx_i_sb[:, mof + k:mof + k + 1],
                                axis=0),
                            in_=data16[:, c0 + k:c0 + k + 1, :].squeeze(1),
                            in_offset=None,
                        ).then_inc(ssem, 16)
                        n_sc += 1
                if g + LOOK < ng:
                    gather(g + LOOK)
            gpsimd.wait_ge(ssem, 16 * n_sc)

    nc.compile()
    return nc


def kernel(input_tensor, expert_indices, expert_mapping):
    global LAST_EXEC_NS, LAST_RESULTS
    from concourse.bass_utils import run_bass_kernel_spmd

    x = np.zeros((T + ZPAD, H), dtype=np.float16)
    x[:T] = np.asarray(input_tensor, dtype=np.float32).reshape(
        T, H).astype(np.float16)
    idx = np.asarray(expert_indices, dtype=np.int32).reshape(-1)
    emap = np.asarray(expert_mapping, dtype=np.int32)
    owner = emap[idx]                                  # [T*K], slot r = t*K+k

    dsts = [np.nonzero(owner == d)[0] for d in range(D)]
    sizes = [len(v) for v in dsts]

    # Smallest uniform per-core chunk count nch such that every slab's
    # overflow (in 128-row export chunks) fits into other cores' spare
    # chunk slots.
    nch = -(-max(TK // D, max(sizes)) // CH)
    for cand in range(-(-(TK // D) // CH), nch + 1):
        spare = sum(max(0, cand - (-(-min(s, cand * CH) // CH)))
                    for s in sizes)
        exp = sum(-(-max(0, s - cand * CH) // CH) for s in sizes)
        if spare >= exp:
            nch = cand
            break
    maxn = nch * CH

    kept = [dsts[d][: min(sizes[d], maxn)] for d in range(D)]
    exports = []                       # (owner, rows) in 128-row chunks
    for d in range(D):
        rest = dsts[d][maxn:]
        for lo in range(0, len(rest), CH):
            exports.append((d, rest[lo: lo + CH]))

    # Assign export chunks to cores with spare chunk slots.  Core 0 hosts
    # only if the others can't absorb everything (trailing-pad slots are
    # skipped via negative indices, so unused spare is free).
    spare_of = [nch - (-(-len(kept[d]) // CH)) for d in range(D)]
    hosted = [[] for _ in range(D)]    # per host core: list of (owner, rows)
    order = sorted(range(1, D), key=lambda d: -spare_of[d]) + [0]
    pos = 0
    for exp in exports:
        while spare_of[order[pos]] - len(hosted[order[pos]]) <= 0:
            pos += 1
        hosted[order[pos]].append(exp)

    if nch not in _CACHE:
        _CACHE[nch] = _build_module(nch)
    nc = _CACHE[nch]

    in_maps = []
    for d in range(D):
        forbid = np.zeros(TK, bool)
        forbid[kept[d]] = True
        for o, rows in hosted[d]:
            forbid[rows] = True
        free_rows = np.nonzero(~forbid)[0]

        # slot sequence: own rows (tail-padded to a chunk boundary), then
        # each hosted export chunk (padded), then all-pad chunks.
        seq_s, seq_t = [], []
        fpos = 0
        seq_s.append(kept[d] // K)
        seq_t.append(kept[d])
        total = len(kept[d])
        if total % CH:
            npad_c = CH - total % CH
            seq_s.append(ZROW + (np.arange(npad_c) % ZPAD))
            seq_t.append(free_rows[fpos:fpos + npad_c])
            fpos += npad_c
            total += npad_c
        for o, rows in hosted[d]:
            seq_s.append(rows // K)
            seq_t.append(rows)
            total += len(rows)
            if len(rows) % CH:
                npad_c = CH - len(rows) % CH
                seq_s.append(ZROW + (np.arange(npad_c) % ZPAD))
                seq_t.append(free_rows[fpos:fpos + npad_c])
                fpos += npad_c
                total += npad_c
        if total < maxn:
            nrest = maxn - total
            seq_s.append(ZROW + (np.arange(nrest) % ZPAD))
            seq_t.append(free_rows[fpos:fpos + nrest])
            fpos += nrest
        srcfull = np.concatenate(seq_s)
        dstfull = np.concatenate(seq_t)
        assert len(srcfull) == maxn

        _, n_add_ch = _group_plan(nch)
        n_ind_ch = nch - n_add_ch
        in_maps.append({
            "xin": x,
            "sidx": _wrap_idxs16(srcfull),
            "didx": _wrap_idxs16(dstfull[:n_add_ch * CH])
            if n_add_ch else np.zeros((128, 16), np.int16),
            "didx_i": np.ascontiguousarray(
                dstfull[n_add_ch * CH:].astype(np.int32).reshape(
                    n_ind_ch, CH).T)
            if n_ind_ch else np.zeros((128, 1), np.int32),
        })

    res = run_bass_kernel_spmd(nc, in_maps, list(range(D)), trace=TRACE)
    if TRACE:
        LAST_EXEC_NS = res.exec_time_ns
        LAST_RESULTS = res
    outs = [np.array(res.results[d]["out"]).astype(np.float32)
            for d in range(D)]
    for c in range(D):
        for o, rows in hosted[c]:
            outs[o][rows] = np.asarray(
                res.results[c]["out"][rows], dtype=np.float32)
            outs[c][rows] = 0.0
    return np.stack(outs, axis=0)
